# revision 22
# baseline (speedup 1.0000x reference)
"""Trainium2 Bass kernel for APGLinear (hypernet-generated per-sample Linear).

Reference computation (B=4096, IN=256, OUT=256, COND=128, HID=512):
    hyp      = relu(condition_z @ W1 + b1)            # [B, HID]
    weight_S = (hyp @ W2 + b2).reshape(B, IN, OUT)    # 1 GB intermediate
    out      = einsum("bi,bio->bo", input_h, weight_S) + bias

Strategy:
  * Shard OUT across the 8 cores (32 columns each). Each core needs only a
    16 MB slice of W2, all of z / input_h, and computes the full hypernet
    activations locally (cheap). No collectives; host concatenates outputs.
  * The big matmul hyp @ W2_slice runs on the TensorEngine in bf16 with
    hyp^T tiles stationary; per-sample weights exist only in PSUM chunks of
    [128 samples, 2 out-cols x 256 in] (o-major layout).
  * The per-sample contraction sum_i input_h[b,i] * wS[b,i,o] is ONE fused
    custom-DVE op per output column (affine_mul_reduce: multiply + row-sum),
    reading wS straight from PSUM in f32 — no ScalarE downcast pass needed.
    (The ISA-level tensor_tensor_reduce would do the same but faults on this
    HW path; the v2mr fallback uses ScalarE-copy + tensor_mul +
    tensor_reduce instead.) The (input_h @ b2 + bias) term comes from a tiny
    augmented matmul and is added at the end.
"""

import os
import sys

import numpy as np

for _p in ("/opt/trn_rl_repo",):
    if os.path.isdir(_p) and _p not in sys.path:
        sys.path.append(_p)

import ml_dtypes  # noqa: E402

import concourse.bass as bass  # noqa: E402,F401
import concourse.tile as tile  # noqa: E402
from concourse import bacc, mybir  # noqa: E402
from concourse.bass_utils import run_bass_kernel_spmd  # noqa: E402

B = 4096
COND = 128
IN = 256
OUT = 256
HID = 512
NCORES = 8
OS = OUT // NCORES  # 32 out-cols per core
KH = HID // 128  # 4 contraction tiles for the big matmul
CH = 512  # psum chunk columns (= one PSUM bank of fp32)
OPC = CH // IN  # out-cols per chunk = 2

MM_DT = mybir.dt.bfloat16
NP_MM_DT = ml_dtypes.bfloat16
F32 = mybir.dt.float32
F16 = mybir.dt.float16


def build(b=B, variant="v10"):
    """Build + compile the SPMD single-core program (same on all 8 cores).

    variant: "v10" (default: v7 + batch-outer hyp loop + c-term in PE tail) |
    "v7" (HW-validated: fused affine_mul_reduce from PSUM,
    deep bufs) | "v2mr" (fallback: ScalarE copy + mul + segmented reduce) |
    "v8" (v2mr + deep bufs) | "v3"/"v6" (coarsened pipelines — slower) |
    "v2ttr"/"v2both" (ISA tensor_tensor_reduce — crashes real HW).
    """
    nt = b // 128  # batch tiles
    hbc = 512  # hyp-phase batch chunk (moving-operand cols)
    nbc = max(1, b // hbc)
    nch = (IN * OS) // CH  # 16 chunks per batch tile

    nc = bacc.Bacc("TRN2", target_bir_lowering=False, debug=False)

    zt_d = nc.dram_tensor("zt", [COND, b], MM_DT, kind="ExternalInput").ap()
    w1_d = nc.dram_tensor("w1", [COND, HID], MM_DT, kind="ExternalInput").ap()
    b1_d = nc.dram_tensor("b1t", [128, KH], F32, kind="ExternalInput").ap()
    # input_h duplicated along cols ([inh, inh]) in bf16, for the fused
    # multiply-reduce against o-major wS chunks
    inh_d = nc.dram_tensor("inhb", [b, CH], MM_DT, kind="ExternalInput").ap()
    ita_d = nc.dram_tensor("inhta", [IN + 128, b], MM_DT, kind="ExternalInput").ap()
    b2a_d = nc.dram_tensor("b2a", [IN + 128, OS], MM_DT, kind="ExternalInput").ap()
    w2_d = nc.dram_tensor("w2s", [HID, IN * OS], MM_DT, kind="ExternalInput").ap()
    out_d = nc.dram_tensor("out", [b, OS], F32, kind="ExternalOutput").ap()

    relu = mybir.ActivationFunctionType.Relu
    mult = mybir.AluOpType.mult
    add = mybir.AluOpType.add

    with tile.TileContext(nc) as tc:
        with tc.tile_pool(name="persist", bufs=1) as pp:
            zt = pp.tile([COND, b], MM_DT)
            nc.sync.dma_start(zt, zt_d)
            w1 = pp.tile([COND, HID], MM_DT)
            nc.sync.dma_start(w1, w1_d)
            b1t = pp.tile([128, KH], F32)
            nc.sync.dma_start(b1t, b1_d)
            ita0 = pp.tile([128, b], MM_DT)
            nc.sync.dma_start(ita0, ita_d[0:128, :])
            ita1 = pp.tile([128, b], MM_DT)
            nc.sync.dma_start(ita1, ita_d[128:256, :])
            ita2 = pp.tile([128, b], MM_DT)
            nc.sync.dma_start(ita2, ita_d[256:384, :])
            b2a0 = pp.tile([128, OS], MM_DT)
            nc.sync.dma_start(b2a0, b2a_d[0:128, :])
            b2a1 = pp.tile([128, OS], MM_DT)
            nc.sync.dma_start(b2a1, b2a_d[128:256, :])
            b2a2 = pp.tile([128, OS], MM_DT)
            nc.sync.dma_start(b2a2, b2a_d[256:384, :])

            inh_all = pp.tile([128, nt * CH], MM_DT)
            for t in range(nt):
                nc.sync.dma_start(
                    inh_all[:, t * CH : (t + 1) * CH], inh_d[t * 128 : (t + 1) * 128, :]
                )

            hyps = [pp.tile([128, b], MM_DT, name=f"hypt{k}") for k in range(KH)]
            oacc = pp.tile([128, nt * OS], F32)
            cterm = pp.tile([128, nt * OS], F32)
            if variant == "pe_act":
                nc.vector.memset(oacc, 0.0)

            # ---- hypernet layer 1: hyp^T = relu(W1^T @ z^T + b1) ----
            hyp_order = (
                [(k, c2) for c2 in range(nbc) for k in range(KH)]
                if variant == "v10"
                else [(k, c2) for k in range(KH) for c2 in range(nbc)]
            )
            with tc.tile_pool(name="hpsum", bufs=4, space="PSUM") as hpp:
                for k, c2 in hyp_order:
                    hp = hpp.tile([128, min(hbc, b)], F32, tag="hp")
                    bs = slice(c2 * hbc, c2 * hbc + min(hbc, b))
                    nc.tensor.matmul(
                        hp,
                        w1[:, k * 128 : (k + 1) * 128],
                        zt[:, bs],
                        start=True,
                        stop=True,
                    )
                    nc.scalar.activation(
                        hyps[k][:, bs], hp, relu, bias=b1t[:, k : k + 1], scale=1.0
                    )
                # ---- c-term: cterm = input_h @ b2_slice + bias_slice ----
                # (for v10 this moves after the main loop, into the PE tail)
                if variant != "v10":
                    for t in range(nt):
                        cp = hpp.tile([128, OS], F32, tag="cp")
                        bsl = slice(t * 128, (t + 1) * 128)
                        nc.tensor.matmul(cp, ita0[:, bsl], b2a0, start=True, stop=False)
                        nc.tensor.matmul(cp, ita1[:, bsl], b2a1, start=False, stop=False)
                        nc.tensor.matmul(cp, ita2[:, bsl], b2a2, start=False, stop=True)
                        nc.vector.tensor_copy(cterm[:, t * OS : (t + 1) * OS], cp)

            # ---- main: wS chunks in PSUM -> bf16 SBUF (ScalarE) -> fused
            # multiply(+reduce) on the Vector engine ----
            scratch = pp.tile([128, CH], MM_DT)
            copyf = mybir.ActivationFunctionType.Copy
            deep = variant in ("v8", "v7", "v9", "v10")
            with (
                tc.tile_pool(name="w2p", bufs=4 if deep else 3) as w2p,
                tc.tile_pool(name="wsp", bufs=8 if deep else 4) as wsp,
                tc.tile_pool(name="mp", bufs=8 if deep else 6, space="PSUM") as mp,
            ):
                oacc3 = oacc.rearrange("p (t o) -> p t o", o=OS)
                if variant == "v6":
                    # k-grouped: one stationary load covers 4 chunk-matmuls
                    # (4 PSUM banks accumulate in parallel per k step)
                    gsz = 4
                    for g in range(nch // gsz):
                        w2g = w2p.tile([128, KH * gsz * CH], MM_DT, tag="w2g")
                        for k in range(KH):
                            nc.sync.dma_start(
                                w2g[:, k * gsz * CH : (k + 1) * gsz * CH],
                                w2_d[
                                    k * 128 : (k + 1) * 128,
                                    g * gsz * CH : (g + 1) * gsz * CH,
                                ],
                            )
                        for t in range(nt):
                            ps4 = mp.tile([128, gsz * CH], F32, tag="ps4", bufs=2)
                            for k in range(KH):
                                for cc in range(gsz):
                                    nc.tensor.matmul(
                                        ps4[:, cc * CH : (cc + 1) * CH],
                                        hyps[k][:, t * 128 : (t + 1) * 128],
                                        w2g[:, (k * gsz + cc) * CH : (k * gsz + cc + 1) * CH],
                                        start=(k == 0),
                                        stop=(k == KH - 1),
                                        skip_group_check=True,
                                    )
                            for cc in range(gsz):
                                c = g * gsz + cc
                                wsb6 = wsp.tile([128, CH], MM_DT, tag="wsb6")
                                nc.scalar.activation(
                                    wsb6, ps4[:, cc * CH : (cc + 1) * CH], copyf
                                )
                                prod6 = wsp.tile([128, CH], MM_DT, tag="prod6")
                                nc.vector.tensor_mul(
                                    prod6, wsb6, inh_all[:, t * CH : (t + 1) * CH]
                                )
                                nc.vector.tensor_reduce(
                                    oacc3[:, t, OPC * c : OPC * (c + 1)],
                                    prod6.rearrange("p (o i) -> p o i", i=IN),
                                    axis=mybir.AxisListType.X,
                                    op=add,
                                )
                    # the shared cterm add below the loop still runs
                    nch_eff = 0
                else:
                    nch_eff = nch
                for c in range(nch_eff):
                    w2c = w2p.tile([128, KH * CH], MM_DT)
                    for k in range(KH):
                        nc.sync.dma_start(
                            w2c[:, k * CH : (k + 1) * CH],
                            w2_d[k * 128 : (k + 1) * 128, c * CH : (c + 1) * CH],
                        )
                    if variant == "v3":
                        # epilogue fused across batch-tile pairs: one mult +
                        # one segmented reduce per 2 chunks of work
                        for tp in range(nt // 2):
                            wsb2 = wsp.tile([128, 2 * CH], MM_DT, tag="wsb2")
                            for tsub in range(2):
                                t = 2 * tp + tsub
                                ps = mp.tile([128, CH], F32)
                                for k in range(KH):
                                    nc.tensor.matmul(
                                        ps,
                                        hyps[k][:, t * 128 : (t + 1) * 128],
                                        w2c[:, k * CH : (k + 1) * CH],
                                        start=(k == 0),
                                        stop=(k == KH - 1),
                                    )
                                nc.scalar.activation(
                                    wsb2[:, tsub * CH : (tsub + 1) * CH], ps, copyf
                                )
                            prod2 = wsp.tile([128, 2 * CH], MM_DT, tag="prod2")
                            nc.vector.tensor_mul(
                                prod2,
                                wsb2,
                                inh_all[:, 2 * tp * CH : (2 * tp + 2) * CH],
                            )
                            nc.vector.tensor_reduce(
                                oacc3[:, 2 * tp : 2 * tp + 2, OPC * c : OPC * (c + 1)],
                                prod2.rearrange("p (t o i) -> p t o i", o=OPC, i=IN),
                                axis=mybir.AxisListType.X,
                                op=add,
                            )
                        continue
                    for t in range(nt):
                        ps = mp.tile([128, CH], F32)
                        for k in range(KH):
                            nc.tensor.matmul(
                                ps,
                                hyps[k][:, t * 128 : (t + 1) * 128],
                                w2c[:, k * CH : (k + 1) * CH],
                                start=(k == 0),
                                stop=(k == KH - 1),
                            )
                        if variant == "v9":
                            # like v7 but via a ScalarE bf16 downcast so the
                            # fused DVE op reads SBUF bf16 (2x-mode eligible)
                            wsb9 = wsp.tile([128, CH], MM_DT, tag="wsb9")
                            nc.scalar.activation(wsb9, ps, copyf)
                            for orel in range(OPC):
                                o = OPC * c + orel
                                nc.vector.affine_mul_reduce(
                                    scratch[:, orel * IN : (orel + 1) * IN],
                                    oacc[:, t * OS + o : t * OS + o + 1],
                                    wsb9[:, orel * IN : (orel + 1) * IN],
                                    inh_all[:, t * CH + orel * IN : t * CH + (orel + 1) * IN],
                                    scale=1.0,
                                    bias=0.0,
                                )
                            continue
                        if variant in ("v7", "v10"):
                            # fused (in0*1+0)*in1 multiply + per-partition sum
                            # on the custom-DVE path, straight from PSUM —
                            # no ScalarE copy, one DVE op per out-col
                            for orel in range(OPC):
                                o = OPC * c + orel
                                nc.vector.affine_mul_reduce(
                                    scratch[:, orel * IN : (orel + 1) * IN],
                                    oacc[:, t * OS + o : t * OS + o + 1],
                                    ps[:, orel * IN : (orel + 1) * IN],
                                    inh_all[:, t * CH + orel * IN : t * CH + (orel + 1) * IN],
                                    scale=1.0,
                                    bias=0.0,
                                )
                            continue
                        wsb = wsp.tile([128, CH], MM_DT)
                        nc.scalar.activation(wsb, ps, copyf)
                        if variant == "pe_act":
                            continue
                        use_ttr = variant == "v2ttr" or (
                            variant == "v2both" and t % 2 == 0
                        )
                        if use_ttr:
                            for orel in range(OPC):
                                o = OPC * c + orel
                                nc.vector.tensor_tensor_reduce(
                                    scratch[:, orel * IN : (orel + 1) * IN],
                                    wsb[:, orel * IN : (orel + 1) * IN],
                                    inh_all[:, t * CH + orel * IN : t * CH + (orel + 1) * IN],
                                    scale=1.0,
                                    scalar=0.0,
                                    op0=mult,
                                    op1=add,
                                    accum_out=oacc[:, t * OS + o : t * OS + o + 1],
                                )
                        else:
                            prod = wsp.tile([128, CH], MM_DT, tag="prod")
                            nc.vector.tensor_mul(
                                prod, wsb, inh_all[:, t * CH : (t + 1) * CH]
                            )
                            nc.vector.tensor_reduce(
                                oacc[:, t * OS + OPC * c : t * OS + OPC * (c + 1)],
                                prod.rearrange("p (o i) -> p o i", i=IN),
                                axis=mybir.AxisListType.X,
                                op=add,
                            )
            if variant == "v10":
                with tc.tile_pool(name="cps", bufs=2, space="PSUM") as cpp:
                    for t in range(nt):
                        cp = cpp.tile([128, OS], F32, tag="cp")
                        bsl = slice(t * 128, (t + 1) * 128)
                        nc.tensor.matmul(cp, ita0[:, bsl], b2a0, start=True, stop=False)
                        nc.tensor.matmul(cp, ita1[:, bsl], b2a1, start=False, stop=False)
                        nc.tensor.matmul(cp, ita2[:, bsl], b2a2, start=False, stop=True)
                        nc.vector.tensor_copy(cterm[:, t * OS : (t + 1) * OS], cp)
            # final: add the (input_h @ b2 + bias) term
            nc.vector.tensor_add(oacc, oacc, cterm)

            for t in range(nt):
                nc.sync.dma_start(
                    out_d[t * 128 : (t + 1) * 128, :], oacc[:, t * OS : (t + 1) * OS]
                )

    nc.compile()
    return nc


def make_in_maps_old(inputs, b=B):
    """Host-side input prep: layout shuffles + dtype casts, per-core shards."""
    inh = np.asarray(inputs["input_h"], dtype=np.float32)
    z = np.asarray(inputs["condition_z"], dtype=np.float32)
    W1 = np.asarray(inputs["W1"], dtype=np.float32)
    b1 = np.asarray(inputs["b1"], dtype=np.float32)
    W2 = np.asarray(inputs["W2"], dtype=np.float32)
    b2 = np.asarray(inputs["b2"], dtype=np.float32)
    bias = np.asarray(inputs["bias"], dtype=np.float32).reshape(1, OUT)

    bf = NP_MM_DT
    zt = np.ascontiguousarray(z.T).astype(bf)  # [COND, b]
    w1h = W1.astype(bf)  # [COND, HID]
    b1t = np.ascontiguousarray(b1.reshape(KH, 128).T).astype(np.float32)  # [128, KH]
    inhb = np.concatenate([inh, inh], axis=1).astype(bf)  # [b, 2*IN]
    # pad the augmented rows to a full 128-partition k-tile (row 256 = ones,
    # rows 257..383 = zeros) so every SBUF tile/matmul uses 128 partitions
    ita = np.concatenate(
        [inh.T, np.ones((1, b), np.float32), np.zeros((127, b), np.float32)], axis=0
    ).astype(bf)
    W2r = W2.reshape(HID, IN, OUT)
    b2r = b2.reshape(IN, OUT)

    in_maps = []
    for c in range(NCORES):
        osl = slice(c * OS, (c + 1) * OS)
        # o-major columns: col = o_rel * IN + i
        w2s = (
            np.ascontiguousarray(W2r[:, :, osl].transpose(0, 2, 1))
            .reshape(HID, OS * IN)
            .astype(bf)
        )
        b2a = np.concatenate(
            [b2r[:, osl], bias[:, osl], np.zeros((127, OS), np.float32)], axis=0
        ).astype(bf)
        in_maps.append(
            {
                "zt": zt,
                "w1": w1h,
                "b1t": b1t,
                "inhb": inhb,
                "inhta": ita,
                "b2a": b2a,
                "w2s": w2s,
            }
        )
    return in_maps


BL = B // NCORES  # 512 local batch rows per core (batch sharding)
NT = BL // 128  # 4 batch tiles per core
NCH = (IN * OUT) // CH  # 128 chunks of 512 W2 columns (2 i-values x 256 o)
IPC = CH // OUT  # i-values per chunk = 2


def build_v11(b=B, variant="v11"):
    """Batch-sharded fused kernel.

    W2 columns stay in their natural order (col = i*OUT + o, o minor), so a
    512-col chunk is exactly {i, i+1} x all 256 outputs. The per-sample
    contraction sum_i x[b,i]*wS[b,i,o] then becomes: ScalarE drains each
    PSUM half-chunk with a fused per-partition scale (x[:, i]) + bf16
    downcast, and the Vector engine folds the scaled pair into two running
    fp32 half-accumulators (even/odd i) with ONE tensor_add per chunk.
    The halves + the (x @ b2 + bias) term are merged in the epilogue.

    variant: "v11" (per-MM stationary reload) | "v12" (k-grouped MMs: one
    hyp^T stationary load covers `GSZ` chunk-matmuls, cutting LDWEIGHTS
    traffic 4x; needs 2 PSUM tiles of GSZ banks).
    """
    nt = NT
    nc = bacc.Bacc("TRN2", target_bir_lowering=False, debug=False)

    zt_d = nc.dram_tensor("zt", [COND, BL], MM_DT, kind="ExternalInput").ap()
    w1_d = nc.dram_tensor("w1", [COND, HID], MM_DT, kind="ExternalInput").ap()
    b1_d = nc.dram_tensor("b1t", [128, KH], F32, kind="ExternalInput").ap()
    xs_d = nc.dram_tensor("xs", [BL, IN], F32, kind="ExternalInput").ap()
    xta_d = nc.dram_tensor("xta", [IN + 128, BL], MM_DT, kind="ExternalInput").ap()
    b2a_d = nc.dram_tensor("b2a", [IN + 128, OUT], MM_DT, kind="ExternalInput").ap()
    w2_d = nc.dram_tensor("w2f", [HID, IN * OUT], MM_DT, kind="ExternalInput").ap()
    out_d = nc.dram_tensor("out", [BL, OUT], F32, kind="ExternalOutput").ap()

    relu = mybir.ActivationFunctionType.Relu
    copyf = mybir.ActivationFunctionType.Copy

    with tile.TileContext(nc) as tc:
        with tc.tile_pool(name="persist", bufs=1) as pp:
            zt = pp.tile([COND, BL], MM_DT)
            nc.sync.dma_start(zt, zt_d)
            w1 = pp.tile([COND, HID], MM_DT)
            nc.sync.dma_start(w1, w1_d)
            b1t = pp.tile([128, KH], F32)
            nc.sync.dma_start(b1t, b1_d)
            xs = pp.tile([128, nt * IN], F32)
            for t in range(nt):
                nc.sync.dma_start(
                    xs[:, t * IN : (t + 1) * IN], xs_d[t * 128 : (t + 1) * 128, :]
                )
            xta = [pp.tile([128, BL], MM_DT, name=f"xta{k}") for k in range(3)]
            b2a = [pp.tile([128, OUT], MM_DT, name=f"b2a{k}") for k in range(3)]
            for k in range(3):
                nc.sync.dma_start(xta[k], xta_d[k * 128 : (k + 1) * 128, :])
                nc.sync.dma_start(b2a[k], b2a_d[k * 128 : (k + 1) * 128, :])

            hyps = [pp.tile([128, BL], MM_DT, name=f"hypt{k}") for k in range(KH)]
            acc_dt = F16 if variant in ("v14", "v17", "v18", "v21", "v22", "v23", "v24") else F32
            tmp_dt = F16 if variant in ("v14", "v17", "v18", "v21", "v22", "v23", "v24") else MM_DT
            acc2 = pp.tile([128, nt * IPC * OUT], acc_dt)
            nc.vector.memset(acc2, 0.0)
            cterm = pp.tile([128, nt * OUT], F32)

            # ---- hypernet layer 1: hyp^T = relu(W1^T @ z^T + b1) ----
            with tc.tile_pool(name="hpsum", bufs=4, space="PSUM") as hpp:
                for k in range(KH):
                    hp = hpp.tile([128, BL], F32, tag="hp")
                    nc.tensor.matmul(
                        hp, w1[:, k * 128 : (k + 1) * 128], zt, start=True, stop=True
                    )
                    nc.scalar.activation(
                        hyps[k], hp, relu, bias=b1t[:, k : k + 1], scale=1.0
                    )
                # ---- c-term: cterm = input_h @ b2 + bias ----
                for t in range(nt):
                    cp = hpp.tile([128, OUT], F32, tag="cp")
                    bsl = slice(t * 128, (t + 1) * 128)
                    nc.tensor.matmul(cp, xta[0][:, bsl], b2a[0], start=True, stop=False)
                    nc.tensor.matmul(cp, xta[1][:, bsl], b2a[1], start=False, stop=False)
                    nc.tensor.matmul(cp, xta[2][:, bsl], b2a[2], start=False, stop=True)
                    nc.vector.tensor_copy(cterm[:, t * OUT : (t + 1) * OUT], cp)

            acc4 = acc2.rearrange("p (t j o) -> p t j o", j=IPC, o=OUT)
            gsz = 4  # v12: chunks per stationary-load group
            w2bufs = 8 if variant in ("v19", "v20", "v21", "v22", "v23", "v24") else 4
            with (
                tc.tile_pool(name="w2p", bufs=w2bufs) as w2p,
                tc.tile_pool(name="tmpp", bufs=8) as tp,
                tc.tile_pool(name="mp", bufs=2 if variant == "v12" else 8,
                             space="PSUM") as mp,
            ):
                if variant == "v12":
                    for g in range(NCH // gsz):
                        w2g = w2p.tile([128, KH * gsz * CH], MM_DT, tag="w2g")
                        for k in range(KH):
                            nc.sync.dma_start(
                                w2g[:, k * gsz * CH : (k + 1) * gsz * CH],
                                w2_d[
                                    k * 128 : (k + 1) * 128,
                                    g * gsz * CH : (g + 1) * gsz * CH,
                                ],
                            )
                        for t in range(nt):
                            ps4 = mp.tile([128, gsz * CH], F32, tag="ps4")
                            for k in range(KH):
                                for cc in range(gsz):
                                    nc.tensor.matmul(
                                        ps4[:, cc * CH : (cc + 1) * CH],
                                        hyps[k][:, t * 128 : (t + 1) * 128],
                                        w2g[:, (k * gsz + cc) * CH : (k * gsz + cc + 1) * CH],
                                        start=(k == 0),
                                        stop=(k == KH - 1),
                                        skip_group_check=True,
                                    )
                            for cc in range(gsz):
                                c = g * gsz + cc
                                tmp = tp.tile([128, CH], MM_DT, tag="tmp")
                                for j in range(IPC):
                                    i = IPC * c + j
                                    nc.scalar.activation(
                                        tmp[:, j * OUT : (j + 1) * OUT],
                                        ps4[:, cc * CH + j * OUT : cc * CH + (j + 1) * OUT],
                                        copyf,
                                        scale=xs[:, t * IN + i : t * IN + i + 1],
                                    )
                                nc.vector.tensor_add(
                                    acc4[:, t],
                                    acc4[:, t],
                                    tmp.rearrange("p (j o) -> p j o", o=OUT),
                                )
                else:
                    mult = mybir.AluOpType.mult
                    add = mybir.AluOpType.add
                    for c in range(NCH):
                        w2c = w2p.tile([128, KH * CH], MM_DT, tag="w2c")
                        if variant in ("v15", "v16", "v20", "v21", "v22", "v23", "v24"):
                            # one 3D-AP DMA per chunk: dst[p, k, col],
                            # src row = k*128 + p of the 512-col slice
                            nc.sync.dma_start(
                                w2c.rearrange("p (k c) -> p k c", k=KH),
                                w2_d[:, c * CH : (c + 1) * CH].rearrange(
                                    "(k p) c -> p k c", k=KH
                                ),
                            )
                        else:
                            for k in range(KH):
                                nc.sync.dma_start(
                                    w2c[:, k * CH : (k + 1) * CH],
                                    w2_d[k * 128 : (k + 1) * 128, c * CH : (c + 1) * CH],
                                )
                        stt_mode = (
                            (variant == "v14" and (c % 5) < 2)
                            or (variant in ("v18", "v22", "v23") and (c % 8) == 4)
                            or (variant == "v24" and (c % 4) == 2)
                        )
                        for t in range(nt):
                            ps = mp.tile([128, CH], F32, tag="ps")
                            for k in range(KH):
                                nc.tensor.matmul(
                                    ps,
                                    hyps[k][:, t * 128 : (t + 1) * 128],
                                    w2c[:, k * CH : (k + 1) * CH],
                                    start=(k == 0),
                                    stop=(k == KH - 1),
                                )
                            if stt_mode:
                                # Vector engine drains PSUM directly:
                                # acc_j = (ps_j * x[:, i_j]) + acc_j
                                for j in range(IPC):
                                    i = IPC * c + j
                                    nc.vector.scalar_tensor_tensor(
                                        acc4[:, t, j],
                                        ps[:, j * OUT : (j + 1) * OUT],
                                        xs[:, t * IN + i : t * IN + i + 1],
                                        acc4[:, t, j],
                                        op0=mult,
                                        op1=add,
                                    )
                                continue
                            tmp = tp.tile([128, CH], tmp_dt, tag="tmp")
                            for j in range(IPC):
                                i = IPC * c + j
                                nc.scalar.activation(
                                    tmp[:, j * OUT : (j + 1) * OUT],
                                    ps[:, j * OUT : (j + 1) * OUT],
                                    copyf,
                                    scale=xs[:, t * IN + i : t * IN + i + 1],
                                )
                            nc.vector.tensor_add(
                                acc4[:, t],
                                acc4[:, t],
                                tmp.rearrange("p (j o) -> p j o", o=OUT),
                            )

            # ---- epilogue: out = acc_even + acc_odd + cterm ----
            with tc.tile_pool(name="op", bufs=2) as op:
                for t in range(nt):
                    ot = op.tile([128, OUT], F32, tag="ot")
                    nc.vector.tensor_add(ot, acc4[:, t, 0], acc4[:, t, 1])
                    nc.vector.tensor_add(ot, ot, cterm[:, t * OUT : (t + 1) * OUT])
                    nc.sync.dma_start(out_d[t * 128 : (t + 1) * 128, :], ot)

    nc.compile()
    return nc


def build_v13(b=B, variant="v13"):
    """Outer-product form: out[b,o] = sum_{h,i} (hyp[b,h]*x[b,i]) * W2r[h,i,o].

    The per-sample contraction disappears into PSUM accumulation: for each
    128-row chunk of the 131072-long (h,i) axis, the Vector engine builds
    P^T[(h,i), b] = x^T[i, b] * hyp^T[h, b] (one bf16 multiply with the hyp
    row partition-broadcast), and the PE runs one n=256 matmul per batch
    tile accumulating into a per-tile PSUM bank. No PSUM drain work at all:
    ScalarE/DVE only touch the 4 final [128, 256] results. The x @ b2 + bias
    term rides along as 3 augmented chunks (x^T rows against b2 rows, ones
    row against the bias row).
    """
    nt = NT
    nkk = HID * 2  # 1024 chunks of 128 (h,i) rows: kk = (h, ihalf)
    nc = bacc.Bacc("TRN2", target_bir_lowering=False, debug=False)

    zt_d = nc.dram_tensor("zt", [COND, BL], MM_DT, kind="ExternalInput").ap()
    w1_d = nc.dram_tensor("w1", [COND, HID], MM_DT, kind="ExternalInput").ap()
    b1_d = nc.dram_tensor("b1t", [128, KH], F32, kind="ExternalInput").ap()
    xta_d = nc.dram_tensor("xta", [IN + 128, BL], MM_DT, kind="ExternalInput").ap()
    b2a_d = nc.dram_tensor("b2a", [IN + 128, OUT], MM_DT, kind="ExternalInput").ap()
    w2_d = nc.dram_tensor("w2f", [HID, IN * OUT], MM_DT, kind="ExternalInput").ap()
    out_d = nc.dram_tensor("out", [BL, OUT], F32, kind="ExternalOutput").ap()

    relu = mybir.ActivationFunctionType.Relu
    copyf = mybir.ActivationFunctionType.Copy

    with tile.TileContext(nc) as tc:
        with tc.tile_pool(name="persist", bufs=1) as pp:
            zt = pp.tile([COND, BL], MM_DT)
            nc.sync.dma_start(zt, zt_d)
            w1 = pp.tile([COND, HID], MM_DT)
            nc.sync.dma_start(w1, w1_d)
            b1t = pp.tile([128, KH], F32)
            nc.sync.dma_start(b1t, b1_d)
            xta = [pp.tile([128, BL], MM_DT, name=f"xta{k}") for k in range(3)]
            b2a = [pp.tile([128, OUT], MM_DT, name=f"b2a{k}") for k in range(3)]
            for k in range(3):
                nc.sync.dma_start(xta[k], xta_d[k * 128 : (k + 1) * 128, :])
                nc.sync.dma_start(b2a[k], b2a_d[k * 128 : (k + 1) * 128, :])

            hyps = [pp.tile([128, BL], MM_DT, name=f"hypt{k}") for k in range(KH)]

            # ---- hypernet layer 1: hyp^T = relu(W1^T @ z^T + b1) ----
            with tc.tile_pool(name="hpsum", bufs=4, space="PSUM") as hpp:
                for k in range(KH):
                    hp = hpp.tile([128, BL], F32, tag="hp")
                    nc.tensor.matmul(
                        hp, w1[:, k * 128 : (k + 1) * 128], zt, start=True, stop=True
                    )
                    nc.scalar.activation(
                        hyps[k], hp, relu, bias=b1t[:, k : k + 1], scale=1.0
                    )

            with (
                tc.tile_pool(name="psf", bufs=1, space="PSUM") as psf,
                tc.tile_pool(name="w2p", bufs=6) as w2p,
                tc.tile_pool(name="ptp", bufs=10) as ptp,
                tc.tile_pool(name="op", bufs=4) as op,
            ):
                pso = [psf.tile([128, OUT], F32, name=f"pso{t}") for t in range(nt)]
                for kk in range(nkk):
                    h, j = kk // 2, kk % 2
                    k, hr = h // 128, h % 128
                    w2c = w2p.tile([128, OUT], MM_DT, tag="w2c")
                    nc.sync.dma_start(
                        w2c,
                        w2_d[h : h + 1, j * 32768 : (j + 1) * 32768].rearrange(
                            "r (i o) -> (r i) o", i=128
                        ),
                    )
                    pt = ptp.tile([128, BL], MM_DT, tag="pt")
                    nc.vector.tensor_mul(
                        pt,
                        xta[j],
                        hyps[k][hr : hr + 1, :].to_broadcast([128, BL]),
                    )
                    for t in range(nt):
                        nc.tensor.matmul(
                            pso[t],
                            pt[:, t * 128 : (t + 1) * 128],
                            w2c,
                            start=(kk == 0),
                            stop=False,
                            skip_group_check=True,
                        )
                # b2 / bias augmented chunks close the accumulation
                for k in range(3):
                    for t in range(nt):
                        nc.tensor.matmul(
                            pso[t],
                            xta[k][:, t * 128 : (t + 1) * 128],
                            b2a[k],
                            start=False,
                            stop=(k == 2),
                            skip_group_check=True,
                        )
                for t in range(nt):
                    ot = op.tile([128, OUT], F32, tag="ot")
                    nc.scalar.activation(ot, pso[t], copyf)
                    nc.sync.dma_start(out_d[t * 128 : (t + 1) * 128, :], ot)

    nc.compile()
    return nc


def make_in_maps_v13(inputs, b=B):
    """Host prep for the outer-product kernel."""
    inh = np.asarray(inputs["input_h"], dtype=np.float32)
    z = np.asarray(inputs["condition_z"], dtype=np.float32)
    W1 = np.asarray(inputs["W1"], dtype=np.float32)
    b1 = np.asarray(inputs["b1"], dtype=np.float32)
    W2 = np.asarray(inputs["W2"], dtype=np.float32)
    b2 = np.asarray(inputs["b2"], dtype=np.float32)
    bias = np.asarray(inputs["bias"], dtype=np.float32).reshape(1, OUT)

    bf = NP_MM_DT
    w1h = W1.astype(bf)
    b1t = np.ascontiguousarray(b1.reshape(KH, 128).T).astype(np.float32)
    w2h = W2.astype(bf)  # [HID, IN*OUT] natural layout
    b2r = b2.reshape(IN, OUT)
    b2a = np.concatenate(
        [b2r, bias, np.zeros((127, OUT), np.float32)], axis=0
    ).astype(bf)

    in_maps = []
    for c in range(NCORES):
        bsl = slice(c * BL, (c + 1) * BL)
        zt = np.ascontiguousarray(z[bsl].T).astype(bf)
        xta = np.concatenate(
            [inh[bsl].T, np.ones((1, BL), np.float32), np.zeros((127, BL), np.float32)],
            axis=0,
        ).astype(bf)
        in_maps.append(
            {"zt": zt, "w1": w1h, "b1t": b1t, "xta": xta, "b2a": b2a, "w2f": w2h}
        )
    return in_maps


def make_in_maps_v11(inputs, b=B):
    """Host-side prep for the batch-sharded kernel. W2 stays in its natural
    layout; each core gets its 512-row batch slice of z / input_h."""
    inh = np.asarray(inputs["input_h"], dtype=np.float32)
    z = np.asarray(inputs["condition_z"], dtype=np.float32)
    W1 = np.asarray(inputs["W1"], dtype=np.float32)
    b1 = np.asarray(inputs["b1"], dtype=np.float32)
    W2 = np.asarray(inputs["W2"], dtype=np.float32)
    b2 = np.asarray(inputs["b2"], dtype=np.float32)
    bias = np.asarray(inputs["bias"], dtype=np.float32).reshape(1, OUT)

    bf = NP_MM_DT
    w1h = W1.astype(bf)
    b1t = np.ascontiguousarray(b1.reshape(KH, 128).T).astype(np.float32)
    w2h = W2.astype(bf)  # [HID, IN*OUT], natural col = i*OUT + o
    b2r = b2.reshape(IN, OUT)
    b2a = np.concatenate(
        [b2r, bias, np.zeros((127, OUT), np.float32)], axis=0
    ).astype(bf)

    in_maps = []
    for c in range(NCORES):
        bsl = slice(c * BL, (c + 1) * BL)
        zt = np.ascontiguousarray(z[bsl].T).astype(bf)  # [COND, BL]
        xs = np.ascontiguousarray(inh[bsl])  # [BL, IN] f32
        xta = np.concatenate(
            [inh[bsl].T, np.ones((1, BL), np.float32), np.zeros((127, BL), np.float32)],
            axis=0,
        ).astype(bf)
        in_maps.append(
            {
                "zt": zt,
                "w1": w1h,
                "b1t": b1t,
                "xs": xs,
                "xta": xta,
                "b2a": b2a,
                "w2f": w2h,
            }
        )
    return in_maps


VARIANT = os.environ.get("APG_VARIANT", "v24")

_NC_CACHE = {}


V11_FAMILY = {"v11", "v12", "v14", "v15", "v16", "v17", "v18", "v19", "v20", "v21", "v22", "v23", "v24"}


def get_nc(b=B, variant=None):
    variant = variant or VARIANT
    key = (b, variant)
    if key not in _NC_CACHE:
        if variant == "v13":
            _NC_CACHE[key] = build_v13(b, variant=variant)
        elif variant in V11_FAMILY:
            _NC_CACHE[key] = build_v11(b, variant=variant)
        else:
            _NC_CACHE[key] = build(b, variant=variant)
    return _NC_CACHE[key]


def make_in_maps(inputs, b=B, variant=None):
    variant = variant or VARIANT
    if variant == "v13":
        return make_in_maps_v13(inputs, b)
    if variant in V11_FAMILY:
        return make_in_maps_v11(inputs, b)
    return make_in_maps_old(inputs, b)


def kernel(**inputs) -> np.ndarray:
    nc = get_nc(B)
    in_maps = make_in_maps(inputs, B)
    res = run_bass_kernel_spmd(nc, in_maps, core_ids=list(range(NCORES)))
    axis = 0 if (VARIANT in V11_FAMILY or VARIANT == "v13") else 1
    out = np.concatenate(
        [res.results[c]["out"] for c in range(NCORES)], axis=axis
    )
    return np.ascontiguousarray(out.astype(np.float32))



# revision 23
# speedup vs baseline: 1.7266x; 1.7266x over previous
"""Trainium2 Bass kernel for APGLinear (hypernet-generated per-sample Linear).

Reference computation (B=4096, IN=256, OUT=256, COND=128, HID=512):
    hyp      = relu(condition_z @ W1 + b1)            # [B, HID]
    weight_S = (hyp @ W2 + b2).reshape(B, IN, OUT)    # 1 GB intermediate
    out      = einsum("bi,bio->bo", input_h, weight_S) + bias

Shipped strategy (default variant "v24", build_v11):
  * Pure data-parallel: batch sharded 8 ways (512 samples/core); W2 is
    replicated and streamed HBM->SBUF once per core (64 MB bf16, one fused
    3D-AP DMA per 512-column chunk, 8-deep prefetch).
  * W2 columns stay in natural order (col = i*OUT + o), so each 512-col
    PSUM chunk is exactly {i, i+1} x all 256 outputs. The TensorEngine runs
    the 2048 [128x128]@[128x512] bf16 matmuls (hyp^T stationary) that
    dominate the roofline (~437 us/core @ 78.6 TF/s).
  * The per-sample contraction sum_i x[b,i]*wS[b,i,o] is split across the
    two non-PE compute engines so neither gates the PE:
      - 3 of 4 chunks: ScalarE drains each PSUM half with a fused
        per-partition scale (x[:, i]) + fp16 downcast; the Vector engine
        folds the pair into two fp16 half-accumulators (even/odd i) with
        one 16-bit-rate tensor_add per chunk-tile.
      - 1 of 4 chunks: the Vector engine drains PSUM directly via
        scalar_tensor_tensor (acc = ps*x + acc), bypassing ScalarE.
  * The (input_h @ b2 + bias) term comes from a tiny augmented matmul; the
    epilogue merges half-accumulators + c-term in fp32 and DMAs out.
  * Old out-sharded variants (build/"v10" etc.) are kept for reference and
    A/B testing via APG_VARIANT.
"""

import os
import sys

import numpy as np

for _p in ("/opt/trn_rl_repo",):
    if os.path.isdir(_p) and _p not in sys.path:
        sys.path.append(_p)

import ml_dtypes  # noqa: E402

import concourse.bass as bass  # noqa: E402,F401
import concourse.tile as tile  # noqa: E402
from concourse import bacc, mybir  # noqa: E402
from concourse.bass_utils import run_bass_kernel_spmd  # noqa: E402

B = 4096
COND = 128
IN = 256
OUT = 256
HID = 512
NCORES = 8
OS = OUT // NCORES  # 32 out-cols per core
KH = HID // 128  # 4 contraction tiles for the big matmul
CH = 512  # psum chunk columns (= one PSUM bank of fp32)
OPC = CH // IN  # out-cols per chunk = 2

MM_DT = mybir.dt.bfloat16
NP_MM_DT = ml_dtypes.bfloat16
F32 = mybir.dt.float32
F16 = mybir.dt.float16


def build(b=B, variant="v10"):
    """Build + compile the SPMD single-core program (same on all 8 cores).

    variant: "v10" (default: v7 + batch-outer hyp loop + c-term in PE tail) |
    "v7" (HW-validated: fused affine_mul_reduce from PSUM,
    deep bufs) | "v2mr" (fallback: ScalarE copy + mul + segmented reduce) |
    "v8" (v2mr + deep bufs) | "v3"/"v6" (coarsened pipelines — slower) |
    "v2ttr"/"v2both" (ISA tensor_tensor_reduce — crashes real HW).
    """
    nt = b // 128  # batch tiles
    hbc = 512  # hyp-phase batch chunk (moving-operand cols)
    nbc = max(1, b // hbc)
    nch = (IN * OS) // CH  # 16 chunks per batch tile

    nc = bacc.Bacc("TRN2", target_bir_lowering=False, debug=False)

    zt_d = nc.dram_tensor("zt", [COND, b], MM_DT, kind="ExternalInput").ap()
    w1_d = nc.dram_tensor("w1", [COND, HID], MM_DT, kind="ExternalInput").ap()
    b1_d = nc.dram_tensor("b1t", [128, KH], F32, kind="ExternalInput").ap()
    # input_h duplicated along cols ([inh, inh]) in bf16, for the fused
    # multiply-reduce against o-major wS chunks
    inh_d = nc.dram_tensor("inhb", [b, CH], MM_DT, kind="ExternalInput").ap()
    ita_d = nc.dram_tensor("inhta", [IN + 128, b], MM_DT, kind="ExternalInput").ap()
    b2a_d = nc.dram_tensor("b2a", [IN + 128, OS], MM_DT, kind="ExternalInput").ap()
    w2_d = nc.dram_tensor("w2s", [HID, IN * OS], MM_DT, kind="ExternalInput").ap()
    out_d = nc.dram_tensor("out", [b, OS], F32, kind="ExternalOutput").ap()

    relu = mybir.ActivationFunctionType.Relu
    mult = mybir.AluOpType.mult
    add = mybir.AluOpType.add

    with tile.TileContext(nc) as tc:
        with tc.tile_pool(name="persist", bufs=1) as pp:
            zt = pp.tile([COND, b], MM_DT)
            nc.sync.dma_start(zt, zt_d)
            w1 = pp.tile([COND, HID], MM_DT)
            nc.sync.dma_start(w1, w1_d)
            b1t = pp.tile([128, KH], F32)
            nc.sync.dma_start(b1t, b1_d)
            ita0 = pp.tile([128, b], MM_DT)
            nc.sync.dma_start(ita0, ita_d[0:128, :])
            ita1 = pp.tile([128, b], MM_DT)
            nc.sync.dma_start(ita1, ita_d[128:256, :])
            ita2 = pp.tile([128, b], MM_DT)
            nc.sync.dma_start(ita2, ita_d[256:384, :])
            b2a0 = pp.tile([128, OS], MM_DT)
            nc.sync.dma_start(b2a0, b2a_d[0:128, :])
            b2a1 = pp.tile([128, OS], MM_DT)
            nc.sync.dma_start(b2a1, b2a_d[128:256, :])
            b2a2 = pp.tile([128, OS], MM_DT)
            nc.sync.dma_start(b2a2, b2a_d[256:384, :])

            inh_all = pp.tile([128, nt * CH], MM_DT)
            for t in range(nt):
                nc.sync.dma_start(
                    inh_all[:, t * CH : (t + 1) * CH], inh_d[t * 128 : (t + 1) * 128, :]
                )

            hyps = [pp.tile([128, b], MM_DT, name=f"hypt{k}") for k in range(KH)]
            oacc = pp.tile([128, nt * OS], F32)
            cterm = pp.tile([128, nt * OS], F32)
            if variant == "pe_act":
                nc.vector.memset(oacc, 0.0)

            # ---- hypernet layer 1: hyp^T = relu(W1^T @ z^T + b1) ----
            hyp_order = (
                [(k, c2) for c2 in range(nbc) for k in range(KH)]
                if variant == "v10"
                else [(k, c2) for k in range(KH) for c2 in range(nbc)]
            )
            with tc.tile_pool(name="hpsum", bufs=4, space="PSUM") as hpp:
                for k, c2 in hyp_order:
                    hp = hpp.tile([128, min(hbc, b)], F32, tag="hp")
                    bs = slice(c2 * hbc, c2 * hbc + min(hbc, b))
                    nc.tensor.matmul(
                        hp,
                        w1[:, k * 128 : (k + 1) * 128],
                        zt[:, bs],
                        start=True,
                        stop=True,
                    )
                    nc.scalar.activation(
                        hyps[k][:, bs], hp, relu, bias=b1t[:, k : k + 1], scale=1.0
                    )
                # ---- c-term: cterm = input_h @ b2_slice + bias_slice ----
                # (for v10 this moves after the main loop, into the PE tail)
                if variant != "v10":
                    for t in range(nt):
                        cp = hpp.tile([128, OS], F32, tag="cp")
                        bsl = slice(t * 128, (t + 1) * 128)
                        nc.tensor.matmul(cp, ita0[:, bsl], b2a0, start=True, stop=False)
                        nc.tensor.matmul(cp, ita1[:, bsl], b2a1, start=False, stop=False)
                        nc.tensor.matmul(cp, ita2[:, bsl], b2a2, start=False, stop=True)
                        nc.vector.tensor_copy(cterm[:, t * OS : (t + 1) * OS], cp)

            # ---- main: wS chunks in PSUM -> bf16 SBUF (ScalarE) -> fused
            # multiply(+reduce) on the Vector engine ----
            scratch = pp.tile([128, CH], MM_DT)
            copyf = mybir.ActivationFunctionType.Copy
            deep = variant in ("v8", "v7", "v9", "v10")
            with (
                tc.tile_pool(name="w2p", bufs=4 if deep else 3) as w2p,
                tc.tile_pool(name="wsp", bufs=8 if deep else 4) as wsp,
                tc.tile_pool(name="mp", bufs=8 if deep else 6, space="PSUM") as mp,
            ):
                oacc3 = oacc.rearrange("p (t o) -> p t o", o=OS)
                if variant == "v6":
                    # k-grouped: one stationary load covers 4 chunk-matmuls
                    # (4 PSUM banks accumulate in parallel per k step)
                    gsz = 4
                    for g in range(nch // gsz):
                        w2g = w2p.tile([128, KH * gsz * CH], MM_DT, tag="w2g")
                        for k in range(KH):
                            nc.sync.dma_start(
                                w2g[:, k * gsz * CH : (k + 1) * gsz * CH],
                                w2_d[
                                    k * 128 : (k + 1) * 128,
                                    g * gsz * CH : (g + 1) * gsz * CH,
                                ],
                            )
                        for t in range(nt):
                            ps4 = mp.tile([128, gsz * CH], F32, tag="ps4", bufs=2)
                            for k in range(KH):
                                for cc in range(gsz):
                                    nc.tensor.matmul(
                                        ps4[:, cc * CH : (cc + 1) * CH],
                                        hyps[k][:, t * 128 : (t + 1) * 128],
                                        w2g[:, (k * gsz + cc) * CH : (k * gsz + cc + 1) * CH],
                                        start=(k == 0),
                                        stop=(k == KH - 1),
                                        skip_group_check=True,
                                    )
                            for cc in range(gsz):
                                c = g * gsz + cc
                                wsb6 = wsp.tile([128, CH], MM_DT, tag="wsb6")
                                nc.scalar.activation(
                                    wsb6, ps4[:, cc * CH : (cc + 1) * CH], copyf
                                )
                                prod6 = wsp.tile([128, CH], MM_DT, tag="prod6")
                                nc.vector.tensor_mul(
                                    prod6, wsb6, inh_all[:, t * CH : (t + 1) * CH]
                                )
                                nc.vector.tensor_reduce(
                                    oacc3[:, t, OPC * c : OPC * (c + 1)],
                                    prod6.rearrange("p (o i) -> p o i", i=IN),
                                    axis=mybir.AxisListType.X,
                                    op=add,
                                )
                    # the shared cterm add below the loop still runs
                    nch_eff = 0
                else:
                    nch_eff = nch
                for c in range(nch_eff):
                    w2c = w2p.tile([128, KH * CH], MM_DT)
                    for k in range(KH):
                        nc.sync.dma_start(
                            w2c[:, k * CH : (k + 1) * CH],
                            w2_d[k * 128 : (k + 1) * 128, c * CH : (c + 1) * CH],
                        )
                    if variant == "v3":
                        # epilogue fused across batch-tile pairs: one mult +
                        # one segmented reduce per 2 chunks of work
                        for tp in range(nt // 2):
                            wsb2 = wsp.tile([128, 2 * CH], MM_DT, tag="wsb2")
                            for tsub in range(2):
                                t = 2 * tp + tsub
                                ps = mp.tile([128, CH], F32)
                                for k in range(KH):
                                    nc.tensor.matmul(
                                        ps,
                                        hyps[k][:, t * 128 : (t + 1) * 128],
                                        w2c[:, k * CH : (k + 1) * CH],
                                        start=(k == 0),
                                        stop=(k == KH - 1),
                                    )
                                nc.scalar.activation(
                                    wsb2[:, tsub * CH : (tsub + 1) * CH], ps, copyf
                                )
                            prod2 = wsp.tile([128, 2 * CH], MM_DT, tag="prod2")
                            nc.vector.tensor_mul(
                                prod2,
                                wsb2,
                                inh_all[:, 2 * tp * CH : (2 * tp + 2) * CH],
                            )
                            nc.vector.tensor_reduce(
                                oacc3[:, 2 * tp : 2 * tp + 2, OPC * c : OPC * (c + 1)],
                                prod2.rearrange("p (t o i) -> p t o i", o=OPC, i=IN),
                                axis=mybir.AxisListType.X,
                                op=add,
                            )
                        continue
                    for t in range(nt):
                        ps = mp.tile([128, CH], F32)
                        for k in range(KH):
                            nc.tensor.matmul(
                                ps,
                                hyps[k][:, t * 128 : (t + 1) * 128],
                                w2c[:, k * CH : (k + 1) * CH],
                                start=(k == 0),
                                stop=(k == KH - 1),
                            )
                        if variant == "v9":
                            # like v7 but via a ScalarE bf16 downcast so the
                            # fused DVE op reads SBUF bf16 (2x-mode eligible)
                            wsb9 = wsp.tile([128, CH], MM_DT, tag="wsb9")
                            nc.scalar.activation(wsb9, ps, copyf)
                            for orel in range(OPC):
                                o = OPC * c + orel
                                nc.vector.affine_mul_reduce(
                                    scratch[:, orel * IN : (orel + 1) * IN],
                                    oacc[:, t * OS + o : t * OS + o + 1],
                                    wsb9[:, orel * IN : (orel + 1) * IN],
                                    inh_all[:, t * CH + orel * IN : t * CH + (orel + 1) * IN],
                                    scale=1.0,
                                    bias=0.0,
                                )
                            continue
                        if variant in ("v7", "v10"):
                            # fused (in0*1+0)*in1 multiply + per-partition sum
                            # on the custom-DVE path, straight from PSUM —
                            # no ScalarE copy, one DVE op per out-col
                            for orel in range(OPC):
                                o = OPC * c + orel
                                nc.vector.affine_mul_reduce(
                                    scratch[:, orel * IN : (orel + 1) * IN],
                                    oacc[:, t * OS + o : t * OS + o + 1],
                                    ps[:, orel * IN : (orel + 1) * IN],
                                    inh_all[:, t * CH + orel * IN : t * CH + (orel + 1) * IN],
                                    scale=1.0,
                                    bias=0.0,
                                )
                            continue
                        wsb = wsp.tile([128, CH], MM_DT)
                        nc.scalar.activation(wsb, ps, copyf)
                        if variant == "pe_act":
                            continue
                        use_ttr = variant == "v2ttr" or (
                            variant == "v2both" and t % 2 == 0
                        )
                        if use_ttr:
                            for orel in range(OPC):
                                o = OPC * c + orel
                                nc.vector.tensor_tensor_reduce(
                                    scratch[:, orel * IN : (orel + 1) * IN],
                                    wsb[:, orel * IN : (orel + 1) * IN],
                                    inh_all[:, t * CH + orel * IN : t * CH + (orel + 1) * IN],
                                    scale=1.0,
                                    scalar=0.0,
                                    op0=mult,
                                    op1=add,
                                    accum_out=oacc[:, t * OS + o : t * OS + o + 1],
                                )
                        else:
                            prod = wsp.tile([128, CH], MM_DT, tag="prod")
                            nc.vector.tensor_mul(
                                prod, wsb, inh_all[:, t * CH : (t + 1) * CH]
                            )
                            nc.vector.tensor_reduce(
                                oacc[:, t * OS + OPC * c : t * OS + OPC * (c + 1)],
                                prod.rearrange("p (o i) -> p o i", i=IN),
                                axis=mybir.AxisListType.X,
                                op=add,
                            )
            if variant == "v10":
                with tc.tile_pool(name="cps", bufs=2, space="PSUM") as cpp:
                    for t in range(nt):
                        cp = cpp.tile([128, OS], F32, tag="cp")
                        bsl = slice(t * 128, (t + 1) * 128)
                        nc.tensor.matmul(cp, ita0[:, bsl], b2a0, start=True, stop=False)
                        nc.tensor.matmul(cp, ita1[:, bsl], b2a1, start=False, stop=False)
                        nc.tensor.matmul(cp, ita2[:, bsl], b2a2, start=False, stop=True)
                        nc.vector.tensor_copy(cterm[:, t * OS : (t + 1) * OS], cp)
            # final: add the (input_h @ b2 + bias) term
            nc.vector.tensor_add(oacc, oacc, cterm)

            for t in range(nt):
                nc.sync.dma_start(
                    out_d[t * 128 : (t + 1) * 128, :], oacc[:, t * OS : (t + 1) * OS]
                )

    nc.compile()
    return nc


def make_in_maps_old(inputs, b=B):
    """Host-side input prep: layout shuffles + dtype casts, per-core shards."""
    inh = np.asarray(inputs["input_h"], dtype=np.float32)
    z = np.asarray(inputs["condition_z"], dtype=np.float32)
    W1 = np.asarray(inputs["W1"], dtype=np.float32)
    b1 = np.asarray(inputs["b1"], dtype=np.float32)
    W2 = np.asarray(inputs["W2"], dtype=np.float32)
    b2 = np.asarray(inputs["b2"], dtype=np.float32)
    bias = np.asarray(inputs["bias"], dtype=np.float32).reshape(1, OUT)

    bf = NP_MM_DT
    zt = np.ascontiguousarray(z.T).astype(bf)  # [COND, b]
    w1h = W1.astype(bf)  # [COND, HID]
    b1t = np.ascontiguousarray(b1.reshape(KH, 128).T).astype(np.float32)  # [128, KH]
    inhb = np.concatenate([inh, inh], axis=1).astype(bf)  # [b, 2*IN]
    # pad the augmented rows to a full 128-partition k-tile (row 256 = ones,
    # rows 257..383 = zeros) so every SBUF tile/matmul uses 128 partitions
    ita = np.concatenate(
        [inh.T, np.ones((1, b), np.float32), np.zeros((127, b), np.float32)], axis=0
    ).astype(bf)
    W2r = W2.reshape(HID, IN, OUT)
    b2r = b2.reshape(IN, OUT)

    in_maps = []
    for c in range(NCORES):
        osl = slice(c * OS, (c + 1) * OS)
        # o-major columns: col = o_rel * IN + i
        w2s = (
            np.ascontiguousarray(W2r[:, :, osl].transpose(0, 2, 1))
            .reshape(HID, OS * IN)
            .astype(bf)
        )
        b2a = np.concatenate(
            [b2r[:, osl], bias[:, osl], np.zeros((127, OS), np.float32)], axis=0
        ).astype(bf)
        in_maps.append(
            {
                "zt": zt,
                "w1": w1h,
                "b1t": b1t,
                "inhb": inhb,
                "inhta": ita,
                "b2a": b2a,
                "w2s": w2s,
            }
        )
    return in_maps


BL = B // NCORES  # 512 local batch rows per core (batch sharding)
NT = BL // 128  # 4 batch tiles per core
NCH = (IN * OUT) // CH  # 128 chunks of 512 W2 columns (2 i-values x 256 o)
IPC = CH // OUT  # i-values per chunk = 2


def build_v11(b=B, variant="v11"):
    """Batch-sharded fused kernel.

    W2 columns stay in their natural order (col = i*OUT + o, o minor), so a
    512-col chunk is exactly {i, i+1} x all 256 outputs. The per-sample
    contraction sum_i x[b,i]*wS[b,i,o] then becomes: ScalarE drains each
    PSUM half-chunk with a fused per-partition scale (x[:, i]) + bf16
    downcast, and the Vector engine folds the scaled pair into two running
    fp32 half-accumulators (even/odd i) with ONE tensor_add per chunk.
    The halves + the (x @ b2 + bias) term are merged in the epilogue.

    variant: "v11" (per-MM stationary reload) | "v12" (k-grouped MMs: one
    hyp^T stationary load covers `GSZ` chunk-matmuls, cutting LDWEIGHTS
    traffic 4x; needs 2 PSUM tiles of GSZ banks).
    """
    nt = NT
    nc = bacc.Bacc("TRN2", target_bir_lowering=False, debug=False)

    zt_d = nc.dram_tensor("zt", [COND, BL], MM_DT, kind="ExternalInput").ap()
    w1_d = nc.dram_tensor("w1", [COND, HID], MM_DT, kind="ExternalInput").ap()
    b1_d = nc.dram_tensor("b1t", [128, KH], F32, kind="ExternalInput").ap()
    xs_d = nc.dram_tensor("xs", [BL, IN], F32, kind="ExternalInput").ap()
    xta_d = nc.dram_tensor("xta", [IN + 128, BL], MM_DT, kind="ExternalInput").ap()
    b2a_d = nc.dram_tensor("b2a", [IN + 128, OUT], MM_DT, kind="ExternalInput").ap()
    w2_d = nc.dram_tensor("w2f", [HID, IN * OUT], MM_DT, kind="ExternalInput").ap()
    out_d = nc.dram_tensor("out", [BL, OUT], F32, kind="ExternalOutput").ap()

    relu = mybir.ActivationFunctionType.Relu
    copyf = mybir.ActivationFunctionType.Copy

    with tile.TileContext(nc) as tc:
        with tc.tile_pool(name="persist", bufs=1) as pp:
            zt = pp.tile([COND, BL], MM_DT)
            nc.sync.dma_start(zt, zt_d)
            w1 = pp.tile([COND, HID], MM_DT)
            nc.sync.dma_start(w1, w1_d)
            b1t = pp.tile([128, KH], F32)
            nc.sync.dma_start(b1t, b1_d)
            xs = pp.tile([128, nt * IN], F32)
            for t in range(nt):
                nc.sync.dma_start(
                    xs[:, t * IN : (t + 1) * IN], xs_d[t * 128 : (t + 1) * 128, :]
                )
            xta = [pp.tile([128, BL], MM_DT, name=f"xta{k}") for k in range(3)]
            b2a = [pp.tile([128, OUT], MM_DT, name=f"b2a{k}") for k in range(3)]
            for k in range(3):
                nc.sync.dma_start(xta[k], xta_d[k * 128 : (k + 1) * 128, :])
                nc.sync.dma_start(b2a[k], b2a_d[k * 128 : (k + 1) * 128, :])

            hyps = [pp.tile([128, BL], MM_DT, name=f"hypt{k}") for k in range(KH)]
            acc_dt = F16 if variant in ("v14", "v17", "v18", "v21", "v22", "v23", "v24") else F32
            tmp_dt = F16 if variant in ("v14", "v17", "v18", "v21", "v22", "v23", "v24") else MM_DT
            acc2 = pp.tile([128, nt * IPC * OUT], acc_dt)
            nc.vector.memset(acc2, 0.0)
            cterm = pp.tile([128, nt * OUT], F32)

            # ---- hypernet layer 1: hyp^T = relu(W1^T @ z^T + b1) ----
            with tc.tile_pool(name="hpsum", bufs=4, space="PSUM") as hpp:
                for k in range(KH):
                    hp = hpp.tile([128, BL], F32, tag="hp")
                    nc.tensor.matmul(
                        hp, w1[:, k * 128 : (k + 1) * 128], zt, start=True, stop=True
                    )
                    nc.scalar.activation(
                        hyps[k], hp, relu, bias=b1t[:, k : k + 1], scale=1.0
                    )
                # ---- c-term: cterm = input_h @ b2 + bias ----
                for t in range(nt):
                    cp = hpp.tile([128, OUT], F32, tag="cp")
                    bsl = slice(t * 128, (t + 1) * 128)
                    nc.tensor.matmul(cp, xta[0][:, bsl], b2a[0], start=True, stop=False)
                    nc.tensor.matmul(cp, xta[1][:, bsl], b2a[1], start=False, stop=False)
                    nc.tensor.matmul(cp, xta[2][:, bsl], b2a[2], start=False, stop=True)
                    nc.vector.tensor_copy(cterm[:, t * OUT : (t + 1) * OUT], cp)

            acc4 = acc2.rearrange("p (t j o) -> p t j o", j=IPC, o=OUT)
            gsz = 4  # v12: chunks per stationary-load group
            w2bufs = 8 if variant in ("v19", "v20", "v21", "v22", "v23", "v24") else 4
            with (
                tc.tile_pool(name="w2p", bufs=w2bufs) as w2p,
                tc.tile_pool(name="tmpp", bufs=8) as tp,
                tc.tile_pool(name="mp", bufs=2 if variant == "v12" else 8,
                             space="PSUM") as mp,
            ):
                if variant == "v12":
                    for g in range(NCH // gsz):
                        w2g = w2p.tile([128, KH * gsz * CH], MM_DT, tag="w2g")
                        for k in range(KH):
                            nc.sync.dma_start(
                                w2g[:, k * gsz * CH : (k + 1) * gsz * CH],
                                w2_d[
                                    k * 128 : (k + 1) * 128,
                                    g * gsz * CH : (g + 1) * gsz * CH,
                                ],
                            )
                        for t in range(nt):
                            ps4 = mp.tile([128, gsz * CH], F32, tag="ps4")
                            for k in range(KH):
                                for cc in range(gsz):
                                    nc.tensor.matmul(
                                        ps4[:, cc * CH : (cc + 1) * CH],
                                        hyps[k][:, t * 128 : (t + 1) * 128],
                                        w2g[:, (k * gsz + cc) * CH : (k * gsz + cc + 1) * CH],
                                        start=(k == 0),
                                        stop=(k == KH - 1),
                                        skip_group_check=True,
                                    )
                            for cc in range(gsz):
                                c = g * gsz + cc
                                tmp = tp.tile([128, CH], MM_DT, tag="tmp")
                                for j in range(IPC):
                                    i = IPC * c + j
                                    nc.scalar.activation(
                                        tmp[:, j * OUT : (j + 1) * OUT],
                                        ps4[:, cc * CH + j * OUT : cc * CH + (j + 1) * OUT],
                                        copyf,
                                        scale=xs[:, t * IN + i : t * IN + i + 1],
                                    )
                                nc.vector.tensor_add(
                                    acc4[:, t],
                                    acc4[:, t],
                                    tmp.rearrange("p (j o) -> p j o", o=OUT),
                                )
                else:
                    mult = mybir.AluOpType.mult
                    add = mybir.AluOpType.add
                    for c in range(NCH):
                        w2c = w2p.tile([128, KH * CH], MM_DT, tag="w2c")
                        if variant in ("v15", "v16", "v20", "v21", "v22", "v23", "v24"):
                            # one 3D-AP DMA per chunk: dst[p, k, col],
                            # src row = k*128 + p of the 512-col slice
                            nc.sync.dma_start(
                                w2c.rearrange("p (k c) -> p k c", k=KH),
                                w2_d[:, c * CH : (c + 1) * CH].rearrange(
                                    "(k p) c -> p k c", k=KH
                                ),
                            )
                        else:
                            for k in range(KH):
                                nc.sync.dma_start(
                                    w2c[:, k * CH : (k + 1) * CH],
                                    w2_d[k * 128 : (k + 1) * 128, c * CH : (c + 1) * CH],
                                )
                        stt_mode = (
                            (variant == "v14" and (c % 5) < 2)
                            or (variant in ("v18", "v22", "v23") and (c % 8) == 4)
                            or (variant == "v24" and (c % 4) == 2)
                        )
                        for t in range(nt):
                            ps = mp.tile([128, CH], F32, tag="ps")
                            for k in range(KH):
                                nc.tensor.matmul(
                                    ps,
                                    hyps[k][:, t * 128 : (t + 1) * 128],
                                    w2c[:, k * CH : (k + 1) * CH],
                                    start=(k == 0),
                                    stop=(k == KH - 1),
                                )
                            if stt_mode:
                                # Vector engine drains PSUM directly:
                                # acc_j = (ps_j * x[:, i_j]) + acc_j
                                for j in range(IPC):
                                    i = IPC * c + j
                                    nc.vector.scalar_tensor_tensor(
                                        acc4[:, t, j],
                                        ps[:, j * OUT : (j + 1) * OUT],
                                        xs[:, t * IN + i : t * IN + i + 1],
                                        acc4[:, t, j],
                                        op0=mult,
                                        op1=add,
                                    )
                                continue
                            tmp = tp.tile([128, CH], tmp_dt, tag="tmp")
                            for j in range(IPC):
                                i = IPC * c + j
                                nc.scalar.activation(
                                    tmp[:, j * OUT : (j + 1) * OUT],
                                    ps[:, j * OUT : (j + 1) * OUT],
                                    copyf,
                                    scale=xs[:, t * IN + i : t * IN + i + 1],
                                )
                            nc.vector.tensor_add(
                                acc4[:, t],
                                acc4[:, t],
                                tmp.rearrange("p (j o) -> p j o", o=OUT),
                            )

            # ---- epilogue: out = acc_even + acc_odd + cterm ----
            with tc.tile_pool(name="op", bufs=2) as op:
                for t in range(nt):
                    ot = op.tile([128, OUT], F32, tag="ot")
                    nc.vector.tensor_add(ot, acc4[:, t, 0], acc4[:, t, 1])
                    nc.vector.tensor_add(ot, ot, cterm[:, t * OUT : (t + 1) * OUT])
                    nc.sync.dma_start(out_d[t * 128 : (t + 1) * 128, :], ot)

    nc.compile()
    return nc


def build_v13(b=B, variant="v13"):
    """Outer-product form: out[b,o] = sum_{h,i} (hyp[b,h]*x[b,i]) * W2r[h,i,o].

    The per-sample contraction disappears into PSUM accumulation: for each
    128-row chunk of the 131072-long (h,i) axis, the Vector engine builds
    P^T[(h,i), b] = x^T[i, b] * hyp^T[h, b] (one bf16 multiply with the hyp
    row partition-broadcast), and the PE runs one n=256 matmul per batch
    tile accumulating into a per-tile PSUM bank. No PSUM drain work at all:
    ScalarE/DVE only touch the 4 final [128, 256] results. The x @ b2 + bias
    term rides along as 3 augmented chunks (x^T rows against b2 rows, ones
    row against the bias row).
    """
    nt = NT
    nkk = HID * 2  # 1024 chunks of 128 (h,i) rows: kk = (h, ihalf)
    nc = bacc.Bacc("TRN2", target_bir_lowering=False, debug=False)

    zt_d = nc.dram_tensor("zt", [COND, BL], MM_DT, kind="ExternalInput").ap()
    w1_d = nc.dram_tensor("w1", [COND, HID], MM_DT, kind="ExternalInput").ap()
    b1_d = nc.dram_tensor("b1t", [128, KH], F32, kind="ExternalInput").ap()
    xta_d = nc.dram_tensor("xta", [IN + 128, BL], MM_DT, kind="ExternalInput").ap()
    b2a_d = nc.dram_tensor("b2a", [IN + 128, OUT], MM_DT, kind="ExternalInput").ap()
    w2_d = nc.dram_tensor("w2f", [HID, IN * OUT], MM_DT, kind="ExternalInput").ap()
    out_d = nc.dram_tensor("out", [BL, OUT], F32, kind="ExternalOutput").ap()

    relu = mybir.ActivationFunctionType.Relu
    copyf = mybir.ActivationFunctionType.Copy

    with tile.TileContext(nc) as tc:
        with tc.tile_pool(name="persist", bufs=1) as pp:
            zt = pp.tile([COND, BL], MM_DT)
            nc.sync.dma_start(zt, zt_d)
            w1 = pp.tile([COND, HID], MM_DT)
            nc.sync.dma_start(w1, w1_d)
            b1t = pp.tile([128, KH], F32)
            nc.sync.dma_start(b1t, b1_d)
            xta = [pp.tile([128, BL], MM_DT, name=f"xta{k}") for k in range(3)]
            b2a = [pp.tile([128, OUT], MM_DT, name=f"b2a{k}") for k in range(3)]
            for k in range(3):
                nc.sync.dma_start(xta[k], xta_d[k * 128 : (k + 1) * 128, :])
                nc.sync.dma_start(b2a[k], b2a_d[k * 128 : (k + 1) * 128, :])

            hyps = [pp.tile([128, BL], MM_DT, name=f"hypt{k}") for k in range(KH)]

            # ---- hypernet layer 1: hyp^T = relu(W1^T @ z^T + b1) ----
            with tc.tile_pool(name="hpsum", bufs=4, space="PSUM") as hpp:
                for k in range(KH):
                    hp = hpp.tile([128, BL], F32, tag="hp")
                    nc.tensor.matmul(
                        hp, w1[:, k * 128 : (k + 1) * 128], zt, start=True, stop=True
                    )
                    nc.scalar.activation(
                        hyps[k], hp, relu, bias=b1t[:, k : k + 1], scale=1.0
                    )

            with (
                tc.tile_pool(name="psf", bufs=1, space="PSUM") as psf,
                tc.tile_pool(name="w2p", bufs=6) as w2p,
                tc.tile_pool(name="ptp", bufs=10) as ptp,
                tc.tile_pool(name="op", bufs=4) as op,
            ):
                pso = [psf.tile([128, OUT], F32, name=f"pso{t}") for t in range(nt)]
                for kk in range(nkk):
                    h, j = kk // 2, kk % 2
                    k, hr = h // 128, h % 128
                    w2c = w2p.tile([128, OUT], MM_DT, tag="w2c")
                    nc.sync.dma_start(
                        w2c,
                        w2_d[h : h + 1, j * 32768 : (j + 1) * 32768].rearrange(
                            "r (i o) -> (r i) o", i=128
                        ),
                    )
                    pt = ptp.tile([128, BL], MM_DT, tag="pt")
                    nc.vector.tensor_mul(
                        pt,
                        xta[j],
                        hyps[k][hr : hr + 1, :].to_broadcast([128, BL]),
                    )
                    for t in range(nt):
                        nc.tensor.matmul(
                            pso[t],
                            pt[:, t * 128 : (t + 1) * 128],
                            w2c,
                            start=(kk == 0),
                            stop=False,
                            skip_group_check=True,
                        )
                # b2 / bias augmented chunks close the accumulation
                for k in range(3):
                    for t in range(nt):
                        nc.tensor.matmul(
                            pso[t],
                            xta[k][:, t * 128 : (t + 1) * 128],
                            b2a[k],
                            start=False,
                            stop=(k == 2),
                            skip_group_check=True,
                        )
                for t in range(nt):
                    ot = op.tile([128, OUT], F32, tag="ot")
                    nc.scalar.activation(ot, pso[t], copyf)
                    nc.sync.dma_start(out_d[t * 128 : (t + 1) * 128, :], ot)

    nc.compile()
    return nc


def make_in_maps_v13(inputs, b=B):
    """Host prep for the outer-product kernel."""
    inh = np.asarray(inputs["input_h"], dtype=np.float32)
    z = np.asarray(inputs["condition_z"], dtype=np.float32)
    W1 = np.asarray(inputs["W1"], dtype=np.float32)
    b1 = np.asarray(inputs["b1"], dtype=np.float32)
    W2 = np.asarray(inputs["W2"], dtype=np.float32)
    b2 = np.asarray(inputs["b2"], dtype=np.float32)
    bias = np.asarray(inputs["bias"], dtype=np.float32).reshape(1, OUT)

    bf = NP_MM_DT
    w1h = W1.astype(bf)
    b1t = np.ascontiguousarray(b1.reshape(KH, 128).T).astype(np.float32)
    w2h = W2.astype(bf)  # [HID, IN*OUT] natural layout
    b2r = b2.reshape(IN, OUT)
    b2a = np.concatenate(
        [b2r, bias, np.zeros((127, OUT), np.float32)], axis=0
    ).astype(bf)

    in_maps = []
    for c in range(NCORES):
        bsl = slice(c * BL, (c + 1) * BL)
        zt = np.ascontiguousarray(z[bsl].T).astype(bf)
        xta = np.concatenate(
            [inh[bsl].T, np.ones((1, BL), np.float32), np.zeros((127, BL), np.float32)],
            axis=0,
        ).astype(bf)
        in_maps.append(
            {"zt": zt, "w1": w1h, "b1t": b1t, "xta": xta, "b2a": b2a, "w2f": w2h}
        )
    return in_maps


def make_in_maps_v11(inputs, b=B):
    """Host-side prep for the batch-sharded kernel. W2 stays in its natural
    layout; each core gets its 512-row batch slice of z / input_h."""
    inh = np.asarray(inputs["input_h"], dtype=np.float32)
    z = np.asarray(inputs["condition_z"], dtype=np.float32)
    W1 = np.asarray(inputs["W1"], dtype=np.float32)
    b1 = np.asarray(inputs["b1"], dtype=np.float32)
    W2 = np.asarray(inputs["W2"], dtype=np.float32)
    b2 = np.asarray(inputs["b2"], dtype=np.float32)
    bias = np.asarray(inputs["bias"], dtype=np.float32).reshape(1, OUT)

    bf = NP_MM_DT
    w1h = W1.astype(bf)
    b1t = np.ascontiguousarray(b1.reshape(KH, 128).T).astype(np.float32)
    w2h = W2.astype(bf)  # [HID, IN*OUT], natural col = i*OUT + o
    b2r = b2.reshape(IN, OUT)
    b2a = np.concatenate(
        [b2r, bias, np.zeros((127, OUT), np.float32)], axis=0
    ).astype(bf)

    in_maps = []
    for c in range(NCORES):
        bsl = slice(c * BL, (c + 1) * BL)
        zt = np.ascontiguousarray(z[bsl].T).astype(bf)  # [COND, BL]
        xs = np.ascontiguousarray(inh[bsl])  # [BL, IN] f32
        xta = np.concatenate(
            [inh[bsl].T, np.ones((1, BL), np.float32), np.zeros((127, BL), np.float32)],
            axis=0,
        ).astype(bf)
        in_maps.append(
            {
                "zt": zt,
                "w1": w1h,
                "b1t": b1t,
                "xs": xs,
                "xta": xta,
                "b2a": b2a,
                "w2f": w2h,
            }
        )
    return in_maps


VARIANT = os.environ.get("APG_VARIANT", "v24")

_NC_CACHE = {}


V11_FAMILY = {"v11", "v12", "v14", "v15", "v16", "v17", "v18", "v19", "v20", "v21", "v22", "v23", "v24"}


def get_nc(b=B, variant=None):
    variant = variant or VARIANT
    key = (b, variant)
    if key not in _NC_CACHE:
        if variant == "v13":
            _NC_CACHE[key] = build_v13(b, variant=variant)
        elif variant in V11_FAMILY:
            _NC_CACHE[key] = build_v11(b, variant=variant)
        else:
            _NC_CACHE[key] = build(b, variant=variant)
    return _NC_CACHE[key]


def make_in_maps(inputs, b=B, variant=None):
    variant = variant or VARIANT
    if variant == "v13":
        return make_in_maps_v13(inputs, b)
    if variant in V11_FAMILY:
        return make_in_maps_v11(inputs, b)
    return make_in_maps_old(inputs, b)


def kernel(**inputs) -> np.ndarray:
    nc = get_nc(B)
    in_maps = make_in_maps(inputs, B)
    res = run_bass_kernel_spmd(nc, in_maps, core_ids=list(range(NCORES)))
    axis = 0 if (VARIANT in V11_FAMILY or VARIANT == "v13") else 1
    out = np.concatenate(
        [res.results[c]["out"] for c in range(NCORES)], axis=axis
    )
    return np.ascontiguousarray(out.astype(np.float32))

